# revision 10
# baseline (speedup 1.0000x reference)
"""Batched pairwise cosine-similarity (correlation) kernel for Trainium2.

Reference computation (per batch b):
    dots  = x[b].T @ x[b]                  # x[b]: [C=256, P=2048]
    norms = sqrt(sum_c x[b,c,p]^2)
    sim   = dots / max(norms[p]*norms[q], 1e-8), diag forced to 1.0

Strategy: data-parallel over batch across 8 NeuronCores (2 batches/core).
Per batch on-chip:
  1. nsq[p] = sum_c x^2 via a ones-matmul (lhsT = ones[128,128]) -- this also
     broadcasts nsq across all 128 partitions for free.
  2. r = 1/sqrt(nsq) (ACT Sqrt + fast DVE reciprocal).
  3. y = x * r  (column pre-scale) so the Gram of y IS the cosine similarity.
  4. Gram(y) via PE matmuls ([128,128] stationary x [128,512] moving, 2 k-tiles
     accumulated in PSUM), PSUM->SBUF copy, diagonal pinned to 1.0 with a
     gpsimd affine_select, 1 MiB DMA per [128,2048] row block.
"""

import os
import sys

for _p in (
    "/root/.axon_site",
    "/root/.axon_site/_ro/trn_rl_repo",
    "/root/.axon_site/_ro/pypackages",
    "/opt/trn_rl_repo",
):
    if os.path.isdir(_p) and _p not in sys.path:
        sys.path.append(_p)

import numpy as np

import bass_rust
import concourse.bass as bass
import concourse.mybir as mybir
import concourse.tile as tile
from concourse.bass_utils import run_bass_kernel_spmd

F32 = mybir.dt.float32
F32R = mybir.dt.float32r
BF16 = mybir.dt.bfloat16

N_CORES = 8
B, C, P = 16, 256, 2048
BPC = B // N_CORES          # batches per core
KT = C // 128               # contraction tiles
MT = P // 128               # output row tiles
NFREE = 512                 # moving free dim per matmul (one PSUM bank)
NT = P // NFREE

# Matmul input dtype: float32r streams at full PE rate (1 cycle/row) for
# moving dims >= 256, unlike float32 (4 cycles/row).
MM_DTYPE = os.environ.get("CORR_MM_DTYPE", "f32r")


def _split_multi_waits(nc: bass.Bass) -> None:
    """Walrus in this container accepts at most ONE sync wait per instruction
    (setupSyncWait raises "Too many sync wait commands" otherwise). Split any
    instruction carrying n>1 waits into (n-1) single-wait NoOps on the same
    engine queue followed by the instruction with its last wait. Engine queues
    dispatch in order, so the gating semantics are preserved.
    """
    ctr = 0
    for f in nc.m.functions:
        for blk in f.blocks:
            new = []
            changed = False
            for inst in blk.instructions:
                si = inst.sync_info
                waits = list(si.on_wait) if si else []
                if len(waits) > 1:
                    changed = True
                    for w in waits[:-1]:
                        ctr += 1
                        nop = mybir.InstNoOp(
                            name=f"waitsplit-{ctr}", ins=[], outs=[]
                        )
                        nop.engine = inst.engine
                        nop.sync_info = bass_rust.SyncInfo(
                            on_wait=[w], on_update=[]
                        )
                        new.append(nop)
                    inst.sync_info = bass_rust.SyncInfo(
                        on_wait=[waits[-1]], on_update=list(si.on_update)
                    )
                new.append(inst)
            if changed:
                blk.instructions = new


def build_kernel(mm_dtype: str = MM_DTYPE, repeat: int = 1) -> bass.Bass:
    nc = bass.Bass("TRN2", target_bir_lowering=False, debug=False, num_devices=1)
    x = nc.dram_tensor("x", [BPC, C, P], F32, kind="ExternalInput").ap()
    out = nc.dram_tensor("out", [BPC, P, P], F32, kind="ExternalOutput").ap()

    with tile.TileContext(nc) as tc:
        with (
            tc.tile_pool(name="xp", bufs=3) as xp,
            tc.tile_pool(name="sqp", bufs=2) as sqp,
            tc.tile_pool(name="nsqp", bufs=1, space="PSUM") as nsqp,
            tc.tile_pool(name="snp", bufs=2) as snp,
            tc.tile_pool(name="rp", bufs=2) as rp,
            tc.tile_pool(name="yp", bufs=4) as yp,
            tc.tile_pool(name="gp", bufs=4, space="PSUM") as gp,
            tc.tile_pool(name="op", bufs=3) as op,
            tc.tile_pool(name="onesp", bufs=1) as onesp,
        ):
            ones = onesp.tile([128, 128], F32)
            nc.gpsimd.memset(ones[:], 1.0)
            fill_one = nc.gpsimd.to_reg(1.0)

            for b in [bb for _ in range(repeat) for bb in range(BPC)]:
                xts = []
                sqs = []
                for k in range(KT):
                    xt = xp.tile([128, P], F32)
                    nc.sync.dma_start(xt[:], x[b, k * 128 : (k + 1) * 128, :])
                    xts.append(xt)
                    sq = sqp.tile([128, P], F32)
                    nc.scalar.activation(
                        sq[:], xt[:], mybir.ActivationFunctionType.Square
                    )
                    sqs.append(sq)

                # nsq[m, p] = sum_c x[c, p]^2, identical across partitions m.
                nsq = nsqp.tile([128, P], F32)
                for j in range(NT):
                    js = slice(j * NFREE, (j + 1) * NFREE)
                    for k in range(KT):
                        nc.tensor.matmul(
                            nsq[:, js],
                            ones[:],
                            sqs[k][:, js],
                            start=(k == 0),
                            stop=(k == KT - 1),
                        )

                snorm = snp.tile([128, P], F32)
                nc.scalar.activation(
                    snorm[:], nsq[:], mybir.ActivationFunctionType.Sqrt
                )
                r = rp.tile([128, P], F32)
                nc.vector.reciprocal(r[:], snorm[:])

                ys = []
                y_dt = {"bf16": BF16, "f32r": F32R, "f32": F32}[mm_dtype]
                for k in range(KT):
                    y = yp.tile([128, P], y_dt)
                    nc.vector.tensor_mul(y[:], xts[k][:], r[:])
                    ys.append(y[:])

                for m in range(MT):
                    ms = slice(m * 128, (m + 1) * 128)
                    o = op.tile([128, P], F32)
                    for j in range(NT):
                        js = slice(j * NFREE, (j + 1) * NFREE)
                        g = gp.tile([128, NFREE], F32)
                        for k in range(KT):
                            nc.tensor.matmul(
                                g[:],
                                ys[k][:, ms],
                                ys[k][:, js],
                                start=(k == 0),
                                stop=(k == KT - 1),
                            )
                        nc.vector.tensor_copy(o[:, js], g[:])
                    # Pin the diagonal block to exactly 1.0:
                    # out[p, q] = (p != q) ? sim : 1.0 on the [128,128] slice.
                    nc.gpsimd.affine_select(
                        out=o[:, ms],
                        in_=o[:, ms],
                        compare_op=mybir.AluOpType.not_equal,
                        fill=fill_one,
                        base=0,
                        pattern=[[-1, 128]],
                        channel_multiplier=1,
                    )
                    nc.sync.dma_start(out[b, ms, :], o[:])
    _split_multi_waits(nc)
    return nc


_CACHE: dict[tuple[str, int], bass.Bass] = {}


def _get_nc(mm_dtype: str = MM_DTYPE, repeat: int = 1) -> bass.Bass:
    key = (mm_dtype, repeat)
    if key not in _CACHE:
        _CACHE[key] = build_kernel(mm_dtype, repeat)
    return _CACHE[key]


def kernel(x: np.ndarray) -> np.ndarray:
    x = np.ascontiguousarray(np.asarray(x), dtype=np.float32)
    assert x.shape == (B, C, P), x.shape
    nc = _get_nc()
    in_maps = [
        {"x": x[c * BPC : (c + 1) * BPC]} for c in range(N_CORES)
    ]
    res = run_bass_kernel_spmd(nc, in_maps, core_ids=list(range(N_CORES)))
    return np.concatenate(
        [res.results[c]["out"] for c in range(N_CORES)], axis=0
    )
